# revision 1
# baseline (speedup 1.0000x reference)
"""Chopfield attention (complex QKV projections + real-part softmax attention)
on 8 Trainium2 NeuronCores.

Math (reference):
    Q = R @ W_Q ; K = Y @ W_K ; V = Y @ W_V          (complex, [4096,1024])
    Z = BETA * Re(conj(Q) @ K^T)                      [4096,4096] real
      = (BETA*Q_re) @ K_re^T + (BETA*Q_im) @ K_im^T
    A = softmax(Z, axis=-1)                           real
    out = A @ V                                       (complex)

Sharding: queries (R rows) and keys (Y rows) are both sharded 8-way.
Each core computes its K/V shard, AllGathers K^T and V, computes local
Q, scores, softmax and A@V for its 512 query rows.

Precision: the softmax is near-one-hot (score std ~2900), so the whole
Q/K score chain must be fp32-accurate. All score-chain matmuls use a
3-pass fp16 hi/lo split (fp16 products are exact on the PE and
accumulate in fp32), which lands within ~2e-3 of a pure-fp32 pipeline.
The V path tolerates fp16 single-pass.
"""

import numpy as np

import concourse.bacc as bacc
import concourse.mybir as mybir
import concourse.tile as tile
from concourse.bass_utils import run_bass_kernel_spmd

BETA = 0.03125
P = 128
FP16 = mybir.dt.float16
FP32 = mybir.dt.float32
X = mybir.AxisListType.X


class Cfg:
    def __init__(self, N=4096, M=4096, D=1024, NC=8):
        self.N, self.M, self.D, self.NC = N, M, D, NC
        self.NL = N // NC          # local query rows
        self.ML = M // NC          # local key rows
        self.DT = D // P           # contraction tiles
        self.QTS = self.NL // P    # local query partition-tiles
        self.MTS = self.ML // P    # local key partition-tiles
        self.DF = min(512, D)      # free-dim chunk for D-wide outputs
        self.DCH = D // self.DF    # chunks of D
        self.MTG = M // P          # global key partition-tiles
        self.KHALF = 2 if self.DT % 2 == 0 else 1   # score K-stream halves
        self.SLOT = D * self.ML    # elements per gathered tensor slot
        # slots: 0..3 = KT(re_h, re_l, im_h, im_l) [D, ML]; 4,5 = V(re, im) [ML, D]
        self.NSLOT = 6


def build(cfg: Cfg, reps: int = 1, no_collective: bool = False, stop_after: str | None = None):
    c = cfg
    nc = bacc.Bacc("TRN2", target_bir_lowering=False, debug=False, num_devices=c.NC)

    def din(name, shape, dt=FP16):
        return nc.dram_tensor(name, shape, dt, kind="ExternalInput")

    # stationary weights [D, D] (column-sliced per output tile at load time)
    # weights host-swizzled to [out_block, partition, in_tile*cols] so every
    # per-output-tile slice is one fully-contiguous DMA
    wq = {}
    for comp in ("re", "im", "s"):
        for lvl in ("h", "l"):
            wq[comp, lvl] = din(f"wq_{comp}_{lvl}", [c.DT, P, c.DT * P])
    wk = {}
    for comp in ("re", "im", "s"):
        for lvl in ("h", "l"):
            wk[comp, lvl] = din(f"wk_{comp}_{lvl}", [c.DT, P, c.DT * P])
    wv = {n: din(f"wv_{n}", [c.DCH, P, c.DT * c.DF]) for n in ("re", "im", "s")}

    # moving operands: R^T and Y^T with hi/lo splits (+re+im sum variants
    # for the Karatsuba complex-product decomposition)
    rt = {}
    yt = {}
    for comp in ("re", "im", "s"):
        for lvl in ("h", "l"):
            rt[comp, lvl] = din(f"rt_{comp}_{lvl}", [P, c.DT * c.NL])
            yt[comp, lvl] = din(f"yt_{comp}_{lvl}", [P, c.DT * c.ML])

    ident = din("ident", [P, P])

    o_re = nc.dram_tensor("o_re", [c.NL, c.D], FP32, kind="ExternalOutput")
    o_im = nc.dram_tensor("o_im", [c.NL, c.D], FP32, kind="ExternalOutput")

    with tile.TileContext(nc) as tc:
        with (
            tc.tile_pool(name="pers", bufs=1) as pers,
            tc.tile_pool(name="ps", bufs=1, space="PSUM") as ps,
            tc.tile_pool(name="dram", bufs=1, space="DRAM") as dram,
        ):
            def emit(rep):
                prp = tc.alloc_tile_pool(name=f"prp{rep}", bufs=1)
                qrt = tc.alloc_tile_pool(name=f"qrt{rep}", bufs=1)
                kvp = tc.alloc_tile_pool(name=f"kvp{rep}", bufs=1)
                # ---------- load Y^T (moving operand for K and V projections)
                yts = {}
                for key, t in yt.items():
                    yts[key] = kvp.tile([P, c.DT * c.ML], FP16, tag=f"yt{key}", name=f"yt_{key[0]}_{key[1]}_{rep}")
                    nc.scalar.dma_start(yts[key][:], t.ap())

                ident_sb = pers.tile([P, P], FP16, tag="ident")
                nc.sync.dma_start(ident_sb[:], ident.ap())

                # collective bounce buffers (flat fp16 element counts)
                agk_in = dram.tile([4 * c.SLOT], FP16)
                agk_out = dram.tile([c.NC * 4 * c.SLOT], FP16, addr_space="Shared")
                agv_in = dram.tile([2 * c.SLOT], FP16)
                agv_out = dram.tile([c.NC * 2 * c.SLOT], FP16, addr_space="Shared")

                # ---------- K^T projection: K^T = W_K^T @ Y^T  (3-pass split)
                # K_re^T = WKre^T@YTre + WKim^T@YTimn ; K_im^T = WKim^T@YTre + WKre^T@YTim

                def proj_qk(w, mov, wtag, mw, pool, out_sb=None, bounce_si=None):
                    """Karatsuba complex projection, 3-pass fp16 split per product.
                    m1 = A_re@W_re, m2 = A_im@W_im, m3 = (A_re+A_im)@(W_re+W_im);
                    out_re = m1 - m2, out_im = m3 - m1 - m2."""
                    for dt_out in range(c.DT):
                        wsl = pool.tile([P, 6 * c.DT * P], FP16, tag=wtag, bufs=2)
                        widx = {("re", "h"): 0, ("re", "l"): 1, ("im", "h"): 2,
                                ("im", "l"): 3, ("s", "h"): 4, ("s", "l"): 5}
                        for (wc, wl), wi in widx.items():
                            nc.sync.dma_start(
                                wsl[:, wi * c.DT * P : (wi + 1) * c.DT * P],
                                w[wc, wl].ap()[dt_out],
                            )

                        def wslice(wc, wl, ki):
                            wi = widx[wc, wl]
                            return wsl[:, wi * c.DT * P + ki * P : wi * c.DT * P + (ki + 1) * P]

                        m = {}
                        for prod, (wc, mc) in enumerate(
                            [("re", "re"), ("im", "im"), ("s", "s")]
                        ):
                            pt = ps.tile([P, 512], FP32, tag="ps", bufs=6)
                            m[prod] = pt[:, :mw]
                            nmm = c.DT * 3
                            i = 0
                            for ki in range(c.DT):
                                for wl, ml in (("h", "h"), ("h", "l"), ("l", "h")):
                                    nc.tensor.matmul(
                                        m[prod],
                                        wslice(wc, wl, ki),
                                        mov[mc, ml][:, ki * mw : ki * mw + mw],
                                        start=(i == 0),
                                        stop=(i == nmm - 1),
                                    )
                                    i += 1
                        # DVE may read only ONE operand from PSUM per inst:
                        # stage m2 in SBUF, then chain single-PSUM subtracts.
                        m2s = pool.tile([P, 512], FP32, tag=wtag + "m2s", bufs=2)
                        nc.vector.tensor_copy(m2s[:, :mw], m[1])
                        dre = pool.tile([P, 512], FP32, tag=wtag + "dre", bufs=2)
                        nc.vector.tensor_sub(dre[:, :mw], m[0], m2s[:, :mw])
                        dim = pool.tile([P, 512], FP32, tag=wtag + "dim", bufs=2)
                        nc.vector.tensor_sub(dim[:, :mw], m[2], m2s[:, :mw])
                        nc.vector.tensor_sub(dim[:, :mw], dim[:, :mw], m[0])
                        for comp, d in (("re", dre), ("im", dim)):
                            if out_sb is not None:
                                hi = out_sb[comp, "h"][:, dt_out * mw : (dt_out + 1) * mw]
                                lo = out_sb[comp, "l"][:, dt_out * mw : (dt_out + 1) * mw]
                            else:
                                hi = pool.tile([P, mw], FP16, tag=wtag + comp + "hi", bufs=2)
                                lo = pool.tile([P, mw], FP16, tag=wtag + comp + "lo", bufs=2)
                            nc.vector.tensor_copy(hi if out_sb is not None else hi[:], d[:, :mw])
                            nc.vector.tensor_sub(lo if out_sb is not None else lo[:], d[:, :mw], hi if out_sb is not None else hi[:])
                            if out_sb is None:
                                for lvl, t in (("h", hi), ("l", lo)):
                                    si = bounce_si[comp, lvl]
                                    dst = agk_in[
                                        si * c.SLOT + dt_out * P * mw : si * c.SLOT + (dt_out + 1) * P * mw
                                    ].rearrange("(p m) -> p m", p=P)
                                    nc.gpsimd.dma_start(dst, t[:])

                kp = tc.alloc_tile_pool(name=f"kp{rep}", bufs=1)
                proj_qk(
                    wk, yts, "wksl", c.ML, kp,
                    bounce_si={("re", "h"): 0, ("re", "l"): 1, ("im", "h"): 2, ("im", "l"): 3},
                )
                kp.release()
                if not no_collective:
                    nc.gpsimd.collective_compute(
                        "AllGather",
                        mybir.AluOpType.bypass,
                        replica_groups=[list(range(c.NC))],
                        ins=[agk_in.opt()],
                        outs=[agk_out.opt()],
                    )

                # R^T loads hoisted here: the scalar DMA ring is idle during the
                # V projection, so Q's moving operands are resident before Q starts.
                rts = {}
                for key, t in rt.items():
                    rts[key] = qrt.tile([P, c.DT * c.NL], FP16, tag=f"rt{key}", name=f"rt_{key[0]}_{key[1]}_{rep}")
                    nc.scalar.dma_start(rts[key][:], t.ap())

                vp = tc.alloc_tile_pool(name=f"vp{rep}", bufs=1)
                # ---------- V projection (single-pass fp16, Karatsuba):
                # m1 = Yre@WVre, m2 = Yim@WVim, m3 = (Yre+Yim)@(WVre+WVim)
                # V_re = m1 - m2 ; V_im = m3 - m1 - m2
                for dch in range(c.DCH):
                    wvsl = vp.tile([P, 3 * c.DT * c.DF], FP16, tag="wvsl", bufs=1)
                    wvidx = {"re": 0, "im": 1, "s": 2}
                    for wn, wi in wvidx.items():
                        nc.sync.dma_start(
                            wvsl[:, wi * c.DT * c.DF : (wi + 1) * c.DT * c.DF],
                            wv[wn].ap()[dch],
                        )
                    for mt in range(c.MTS):
                        m = {}
                        for prod, yc in enumerate(("re", "im", "s")):
                            pt = ps.tile([P, 512], FP32, tag="ps", bufs=6)
                            m[prod] = pt[:, : c.DF]
                            wn = yc
                            for ki in range(c.DT):
                                nc.tensor.matmul(
                                    m[prod],
                                    yts[yc, "h"][:, ki * c.ML + mt * P : ki * c.ML + (mt + 1) * P],
                                    wvsl[:, wvidx[wn] * c.DT * c.DF + ki * c.DF : wvidx[wn] * c.DT * c.DF + (ki + 1) * c.DF],
                                    start=(ki == 0),
                                    stop=(ki == c.DT - 1),
                                )
                        vm2s = vp.tile([P, c.DF], FP32, tag="vm2s", bufs=2)
                        nc.vector.tensor_copy(vm2s[:], m[1])
                        for comp, si in (("re", 0), ("im", 1)):
                            vout = vp.tile([P, c.DF], FP16, tag="vout", bufs=4)
                            if comp == "re":
                                nc.vector.tensor_sub(vout[:], m[0], vm2s[:])
                            else:
                                vim1 = vp.tile([P, c.DF], FP32, tag="vim1", bufs=2)
                                nc.vector.tensor_sub(vim1[:], m[2], vm2s[:])
                                nc.vector.tensor_sub(vout[:], vim1[:], m[0])
                            dst = agv_in[si * c.SLOT : (si + 1) * c.SLOT].rearrange(
                                "(m p dc d) -> m p dc d", m=c.MTS, p=P, dc=c.DCH
                            )[mt, :, dch, :]
                            nc.gpsimd.dma_start(dst, vout[:])

                # ---------- AllGather V (A@V consumes it much later)
                if not no_collective:
                    nc.gpsimd.collective_compute(
                        "AllGather",
                        mybir.AluOpType.bypass,
                        replica_groups=[list(range(c.NC))],
                        ins=[agv_in.opt()],
                        outs=[agv_out.opt()],
                    )
                if stop_after == "vproj":
                    vp.release()
                    kvp.release()
                    qrt.release()
                    prp.release()
                    return

                vp.release()
                kvp.release()

                # ---------- Q^T projection (R^T loads hoisted above, before V proj)
                qp = tc.alloc_tile_pool(name=f"qp{rep}", bufs=1)
                qt_sb = {}
                for comp in ("re", "im"):
                    for lvl in ("h", "l"):
                        qt_sb[comp, lvl] = prp.tile([P, c.DT * c.NL], FP16, tag=f"qt{comp}{lvl}", name=f"qt_{comp}_{lvl}_{rep}")
                proj_qk(wq, rts, "wqsl", c.NL, qp, out_sb=qt_sb)
                qp.release()
                qrt.release()
                if stop_after == "qproj":
                    prp.release()
                    return

                # ---------- scores + streaming softmax (per key-shard chunk)
                # Z[q, m] = QT^T @ KT ; chunk max -> exp(Z - cmax); rescale later.
                scp = tc.alloc_tile_pool(name=f"scp{rep}", bufs=1)
                p_sb = [prp.tile([P, c.M], FP16, tag=f"p{qt}", name=f"p_{qt}_{rep}") for qt in range(c.QTS)]
                cm = [prp.tile([P, c.NC], FP32, tag=f"cm{qt}", name=f"cm_{qt}_{rep}") for qt in range(c.QTS)]
                ncm = [prp.tile([P, c.NC], FP32, tag=f"ncm{qt}", name=f"ncm_{qt}_{rep}") for qt in range(c.QTS)]

                kdh = c.DT // c.KHALF  # d-tiles per streamed half
                for r in range(c.NC):
                    halves = []
                    for h in range(c.KHALF):
                        ktl = scp.tile([P, 4 * kdh * c.ML], FP16, tag="ktl", bufs=3)
                        for si in range(4):
                            src = agk_out[
                                r * 4 * c.SLOT
                                + si * c.SLOT
                                + h * kdh * P * c.ML : r * 4 * c.SLOT
                                + si * c.SLOT
                                + (h + 1) * kdh * P * c.ML
                            ].rearrange("(t p m) -> p t m", p=P, m=c.ML)
                            nc.scalar.dma_start(
                                ktl[:, si * kdh * c.ML : (si + 1) * kdh * c.ML].rearrange(
                                    "p (t m) -> p t m", m=c.ML
                                ),
                                src,
                            )
                        halves.append(ktl)

                    def ktslice(comp, lvl, ki):
                        si = {("re", "h"): 0, ("re", "l"): 1, ("im", "h"): 2, ("im", "l"): 3}[comp, lvl]
                        t = halves[ki // kdh]
                        k = ki % kdh
                        return t[:, si * kdh * c.ML + k * c.ML : si * kdh * c.ML + (k + 1) * c.ML]

                    for qt in range(c.QTS):
                        zp = ps.tile([P, 512], FP32, tag="ps", bufs=6)
                        zacc = zp[:, : c.ML]
                        nmm = 2 * c.DT * 3
                        i = 0
                        for comp in ("re", "im"):
                            for ki in range(c.DT):
                                for ql, kl in (("h", "h"), ("h", "l"), ("l", "h")):
                                    nc.tensor.matmul(
                                        zacc,
                                        qt_sb[comp, ql][:, ki * c.NL + qt * P : ki * c.NL + (qt + 1) * P],
                                        ktslice(comp, kl, ki),
                                        start=(i == 0),
                                        stop=(i == nmm - 1),
                                    )
                                    i += 1
                        nc.vector.reduce_max(cm[qt][:, r : r + 1], zacc, axis=X)
                        nc.vector.tensor_scalar_mul(
                            ncm[qt][:, r : r + 1], cm[qt][:, r : r + 1], -1.0
                        )
                        nc.scalar.activation(
                            p_sb[qt][:, r * c.ML : (r + 1) * c.ML],
                            zacc,
                            mybir.ActivationFunctionType.Exp,
                            bias=ncm[qt][:, r : r + 1],
                            scale=1.0,
                        )

                # ---------- finalize softmax: rescale chunks to the global max
                recip = []
                for qt in range(c.QTS):
                    ngm = prp.tile([P, 1], FP32, tag=f"ngm{qt}")
                    nc.vector.tensor_reduce(
                        ngm[:], ncm[qt][:], op=mybir.AluOpType.min, axis=X
                    )
                    fac = prp.tile([P, c.NC], FP32, tag=f"fac{qt}")
                    nc.scalar.activation(
                        fac[:],
                        ncm[qt][:],
                        mybir.ActivationFunctionType.Exp,
                        bias=ngm[:, 0:1],
                        scale=-1.0,
                    )
                    for r in range(c.NC):
                        nc.vector.tensor_scalar_mul(
                            p_sb[qt][:, r * c.ML : (r + 1) * c.ML],
                            p_sb[qt][:, r * c.ML : (r + 1) * c.ML],
                            fac[:, r : r + 1],
                        )
                    ssum = prp.tile([P, 1], FP32, tag=f"ssum{qt}")
                    nc.vector.reduce_sum(ssum[:], p_sb[qt][:], axis=X)
                    rc = prp.tile([P, 1], FP32, tag=f"rcp{qt}")
                    nc.vector.reciprocal(rc[:], ssum[:])
                    recip.append(rc)

                scp.release()
                if stop_after == "scores":
                    prp.release()
                    return

                # ---------- transpose P -> P^T tiles ([m-part, q-free])
                avp = tc.alloc_tile_pool(name=f"avp{rep}", bufs=1)
                pt_sb = [avp.tile([P, c.NL], FP16, tag=f"pt{mtg}", name=f"pt_{mtg}_{rep}") for mtg in range(c.MTG)]
                for mtg in range(c.MTG):
                    tp = ps.tile([P, 512], FP16, tag="dsc", bufs=2)
                    tacc = tp[:, : c.NL]
                    for qt in range(c.QTS):
                        nc.tensor.matmul(
                            tacc[:, qt * P : (qt + 1) * P],
                            p_sb[qt][:, mtg * P : (mtg + 1) * P],
                            ident_sb[:],
                            start=True,
                            stop=True,
                            is_transpose=True,
                        )
                    nc.vector.tensor_copy(pt_sb[mtg][:], tacc)
                if stop_after == "transp":
                    avp.release()
                    prp.release()
                    return

                # ---------- A @ V (+ 1/sum scaling)
                for comp, odram in (("re", o_re), ("im", o_im)):
                    si = 0 if comp == "re" else 1
                    for dch in range(c.DCH):
                        vh = avp.tile([P, c.MTG * c.DF], FP16, tag="vh", bufs=2)
                        for r in range(c.NC):
                            src = agv_out[
                                r * 2 * c.SLOT + si * c.SLOT : r * 2 * c.SLOT + (si + 1) * c.SLOT
                            ].rearrange("(m p dc d) -> dc p m d", m=c.MTS, p=P, dc=c.DCH)[dch]
                            nc.sync.dma_start(
                                vh[
                                    :, r * c.MTS * c.DF : (r + 1) * c.MTS * c.DF
                                ].rearrange("p (m d) -> p m d", m=c.MTS),
                                src,
                            )
                        for qt in range(c.QTS):
                            op_ = ps.tile([P, 512], FP32, tag="ps", bufs=6)
                            oacc = op_[:, : c.DF]
                            for mtg in range(c.MTG):
                                nc.tensor.matmul(
                                    oacc,
                                    pt_sb[mtg][:, qt * P : (qt + 1) * P],
                                    vh[:, mtg * c.DF : (mtg + 1) * c.DF],
                                    start=(mtg == 0),
                                    stop=(mtg == c.MTG - 1),
                                )
                            osb = avp.tile([P, c.DF], FP32, tag="osb", bufs=4)
                            nc.vector.tensor_scalar_mul(osb[:], oacc, recip[qt][:, 0:1])
                            nc.sync.dma_start(
                                odram.ap()[
                                    qt * P : (qt + 1) * P, dch * c.DF : (dch + 1) * c.DF
                                ],
                                osb[:],
                            )
                avp.release()
                prp.release()

            for rep in range(reps):
                emit(rep)

    nc.compile()
    return nc


def _split16(x):
    h = x.astype(np.float16)
    l = (x - h.astype(np.float32)).astype(np.float16)
    return h, l


def prep_inputs(cfg, R_re, R_im, Y_re, Y_im, W_Q_re, W_Q_im, W_K_re, W_K_im, W_V_re, W_V_im):
    """Host-side sharding + fp16 hi/lo split + transposes. Returns in_maps."""
    c = cfg
    f32 = np.float32
    wq_re = np.ascontiguousarray(W_Q_re, dtype=f32) * BETA
    wq_im = np.ascontiguousarray(W_Q_im, dtype=f32) * BETA
    wk_re = np.ascontiguousarray(W_K_re, dtype=f32)
    wk_im = np.ascontiguousarray(W_K_im, dtype=f32)
    wv_re = np.ascontiguousarray(W_V_re, dtype=f32)
    wv_im = np.ascontiguousarray(W_V_im, dtype=f32)
    wqs = {"re": _split16(wq_re), "im": _split16(wq_im), "s": _split16(wq_re + wq_im)}
    wks = {"re": _split16(wk_re), "im": _split16(wk_im), "s": _split16(wk_re + wk_im)}
    ident = np.eye(P, dtype=np.float16)

    DT, DCH, DF = cfg.DT, cfg.DCH, cfg.DF

    def _wsw(w16, ocols):
        # [d_in, d_out] -> [d_out_block, p, d_in_tile * ocols], contiguous
        ob = w16.shape[1] // ocols
        return np.ascontiguousarray(
            w16.reshape(DT, P, ob, ocols).transpose(2, 1, 0, 3).reshape(ob, P, DT * ocols)
        )

    shared = {}
    for comp in ("re", "im", "s"):
        for li, lvl in enumerate(("h", "l")):
            shared[f"wq_{comp}_{lvl}"] = _wsw(wqs[comp][li], P)
            shared[f"wk_{comp}_{lvl}"] = _wsw(wks[comp][li], P)
    shared["wv_re"] = _wsw(wv_re.astype(np.float16), DF)
    shared["wv_im"] = _wsw(wv_im.astype(np.float16), DF)
    shared["wv_s"] = _wsw((wv_re + wv_im).astype(np.float16), DF)
    shared["ident"] = ident

    in_maps = []
    for r in range(c.NC):
        m = dict(shared)
        rsl = slice(r * c.NL, (r + 1) * c.NL)
        ysl = slice(r * c.ML, (r + 1) * c.ML)
        rre_t = np.ascontiguousarray(np.asarray(R_re[rsl], dtype=f32).T)
        rim_t = np.ascontiguousarray(np.asarray(R_im[rsl], dtype=f32).T)
        yre_t = np.ascontiguousarray(np.asarray(Y_re[ysl], dtype=f32).T)
        yim_t = np.ascontiguousarray(np.asarray(Y_im[ysl], dtype=f32).T)
        for base, arr in (("rt_re", rre_t), ("rt_im", rim_t), ("rt_s", rre_t + rim_t),
                          ("yt_re", yre_t), ("yt_im", yim_t), ("yt_s", yre_t + yim_t)):
            h, l = _split16(arr)
            mw = arr.shape[1]
            for lvl, a in (("h", h), ("l", l)):
                m[f"{base}_{lvl}"] = np.ascontiguousarray(
                    a.reshape(DT, P, mw).transpose(1, 0, 2).reshape(P, DT * mw)
                )
        in_maps.append(m)
    return in_maps


_NC_CACHE = {}


def kernel(**inputs) -> np.ndarray:
    cfg = Cfg()
    if "full" not in _NC_CACHE:
        _NC_CACHE["full"] = build(cfg, 1)
    nc = _NC_CACHE["full"]
    in_maps = prep_inputs(cfg, **inputs)
    res = run_bass_kernel_spmd(nc, in_maps, list(range(cfg.NC)))
    o_re = np.concatenate([res.results[r]["o_re"] for r in range(cfg.NC)], axis=0)
    o_im = np.concatenate([res.results[r]["o_im"] for r in range(cfg.NC)], axis=0)
    return (o_re + 1j * o_im).astype(np.complex64)

